# revision 1
# baseline (speedup 1.0000x reference)
"""Trainium2 Bass kernel for nn_GammaSpaceLayer.

The reference is an SSM: fixed "gamma" transition A (bidiagonal), bilinear
discretization, kernel k[m] = C dA^m dB, then FFT causal conv + D*u skip.
This is computed here as a chunked linear scan (state dim H=64, chunk T=8):

  per (batch, chunk c, local t):  x = sum_{s<=t} dA^{t-s} dB u[c,s]  (intra)
                                  + dA^{t+1} xend[c-1]               (inter)
  y = C x + D*u

All heavy work is matmuls on the PE array in a transposed layout
(contraction dims on SBUF partitions).  Data-parallel over batch: 16
batches over 8 cores = 2 per core.  Small input-dependent matrices
(powers of dA) are precomputed on host in float64 and passed as inputs,
so the Bass program is input-independent (NEFF cache friendly).
"""

import numpy as np

import concourse.bass as bass
import concourse.mybir as mybir
import concourse.tile as tile
from concourse.vector_clock import ScopedClock
from concourse.bass_utils import run_bass_kernel_spmd

# problem constants (hardcoded per contract)
H, S = 64, 128          # state dim, io channel dim
B, L = 16, 2048         # full batch, seq len
N_CORES = 8
PB = B // N_CORES       # batches per core (2)
T = 8                   # chunk length
C = L // T              # chunks per batch (256)
SCAN_RADIX = 4          # radix-4 Hillis-Steele scan over chunks
SCAN_LEVELS = 4         # 4^4 = 256 = C
NWP = SCAN_LEVELS * (SCAN_RADIX - 1)   # 12 scan weight matrices
COLS = PB * C           # (b, c) columns per core (512)
DT_MIN, DT_MAX = 0.001, 0.1

F32 = mybir.dt.float32
MM = mybir.dt.float32r  # matmul operand dtype: fp32 bits, fast PE mode (1cy/row at N>=256)


class _TC(tile.TileContext):
    """TileContext whose tail drain splits multi-sem waits: this walrus
    build caps CTRL instructions at one sync-wait command."""

    def _drain_and_barrier(self, tick_clock, wait_clock):
        probe = self.nc.sync.drain()
        wait_clock.add_sem_waits(probe.ins, ScopedClock({None: tick_clock.global_clock}))
        si = probe.ins.sync_info
        if si is not None and si.on_wait and len(si.on_wait) > 1:
            waits = list(si.on_wait)
            probe.ins.sync_info = mybir.SyncInfo(
                on_wait=[waits[0]], on_update=list(si.on_update or []))
            for w in waits[1:]:
                d = self.nc.sync.drain()
                d.ins.sync_info = mybir.SyncInfo(on_wait=[w], on_update=[])
        self.nc.all_engine_barrier()
        assert self.sems is not None
        popped = self.nc._tile_sem_poison_stack.pop()
        assert popped is self._sem_poison
        self.nc.clear_and_free_semaphores(list(self.sems.allocated().values()))
        self.nc.all_engine_barrier()


def _split_multi_waits(nc):
    """This walrus build allows only ONE sync-wait command per instruction.
    Split extras onto same-engine InstEventSemaphore carriers inserted
    immediately before (engine program order preserves semantics)."""
    n = 0
    for f in nc.m.functions:
        for b in f.blocks:
            il = b.instructions
            i = 0
            while i < len(il):
                ins = il[i]
                si = ins.sync_info
                if si is not None and si.on_wait and len(si.on_wait) > 1:
                    waits = list(si.on_wait)
                    ins.sync_info = mybir.SyncInfo(
                        on_wait=[waits[-1]], on_update=list(si.on_update or []))
                    for j, w in enumerate(waits[:-1]):
                        ev = mybir.InstEventSemaphore(
                            name=f"{ins.name}_wsplit{j}", ins=[], outs=[])
                        ev.engine = ins.engine
                        ev.sync_info = mybir.SyncInfo(on_wait=[w], on_update=[])
                        il.insert(i, ev)
                        i += 1
                        n += 1
                i += 1
    return n


def _build():
    nc = bass.Bass()
    u_d = nc.dram_tensor("u", [PB, L, S], MM, kind="ExternalInput")
    gt_d = nc.dram_tensor("GT", [T, S, H], MM, kind="ExternalInput")      # (dA^m dB)^T
    ap_d = nc.dram_tensor("APOWT", [T, H, H], MM, kind="ExternalInput")   # (dA^{t+1})^T
    wp_d = nc.dram_tensor("WPT", [NWP, H, H], MM, kind="ExternalInput")   # scan weights^T
    ct_d = nc.dram_tensor("CT2", [H, S], MM, kind="ExternalInput")        # C^T
    dd_d = nc.dram_tensor("DD", [S, S], MM, kind="ExternalInput")         # diag(D)
    eye_d = nc.dram_tensor("EYE", [128, 128], MM, kind="ExternalInput")
    zz_d = nc.dram_tensor("ZZ", [H, COLS], MM, kind="ExternalInput")      # zeros
    y_d = nc.dram_tensor("y", [PB, L, S], F32, kind="ExternalOutput")

    NTILE = PB * L // 128  # 32 row-tiles of u/y

    with _TC(nc) as tc:
        with (
            tc.tile_pool(name="const", bufs=1) as cpool,
            tc.tile_pool(name="big", bufs=1) as bigpool,
            tc.tile_pool(name="stage", bufs=6) as stpool,
            tc.tile_pool(name="ostage", bufs=6) as ostpool,
            tc.tile_pool(name="psum_t", bufs=3, space="PSUM") as pst,
            tc.tile_pool(name="psum_z", bufs=3, space="PSUM") as psz,
            tc.tile_pool(name="psum_s", bufs=1, space="PSUM") as pss,
            tc.tile_pool(name="psum_y", bufs=1, space="PSUM") as psy,
        ):
            ident = cpool.tile([128, 128], MM)
            nc.sync.dma_start(ident[:], eye_d[:])

            gt_sb = cpool.tile([S, T * H], MM)
            nc.sync.dma_start(gt_sb[:].rearrange("p (m n) -> p m n", m=T),
                              gt_d[:].rearrange("m p n -> p m n"))
            ap_sb = cpool.tile([H, T * H], MM)
            nc.sync.dma_start(ap_sb[:].rearrange("p (m n) -> p m n", m=T),
                              ap_d[:].rearrange("m p n -> p m n"))
            wp_sb = cpool.tile([H, NWP * H], MM)
            nc.sync.dma_start(wp_sb[:].rearrange("p (m n) -> p m n", m=NWP),
                              wp_d[:].rearrange("m p n -> p m n"))
            ct_sb = cpool.tile([H, S], MM)
            nc.sync.dma_start(ct_sb[:], ct_d[:])
            dd_sb = cpool.tile([S, S], MM)
            nc.sync.dma_start(dd_sb[:], dd_d[:])

            # ---- load u and transpose on chip:  uT[i, b*L + l] ----
            uT = bigpool.tile([S, PB * L], MM)
            u_flat = u_d[:].rearrange("b l i -> (b l) i")
            for it in range(NTILE):
                st = stpool.tile([128, S], MM, tag="ustage")
                nc.sync.dma_start(st[:], u_flat[it * 128:(it + 1) * 128, :])
                pt = pst.tile([S, 128], MM, tag="tp")
                nc.tensor.transpose(pt[:], st[:], ident[:])
                eng = nc.scalar.copy if it % 2 else nc.vector.tensor_copy
                eng(uT[:, it * 128:(it + 1) * 128], pt[:])

            uT_r = uT[:].rearrange("p (b c t) -> p b c t", b=PB, c=C, t=T)

            # ---- stage 1 (t = T-1 only): b_c = ZT[T-1] ----
            zlast = psz.tile([H, COLS], F32, tag="z")
            for s in range(T):
                nc.tensor.matmul(
                    zlast[:], gt_sb[:, (T - 1 - s) * H:(T - s) * H], uT_r[:, :, :, s],
                    start=(s == 0), stop=(s == T - 1))

            # ---- chunk-state log-scan (Hillis-Steele with matrix weights) ----
            # layout per batch: [C zero cols | C data cols]
            sc_a = bigpool.tile([H, 2 * PB * C], MM)
            sc_b = bigpool.tile([H, 2 * PB * C], MM)
            sa_r = sc_a[:].rearrange("p (b x) -> p b x", b=PB)
            sb_r = sc_b[:].rearrange("p (b x) -> p b x", b=PB)
            nc.sync.dma_start(sa_r[:, :, 0:C], zz_d[:].rearrange("p (b c) -> p b c", b=PB))
            nc.sync.dma_start(sb_r[:, :, 0:C], zz_d[:].rearrange("p (b c) -> p b c", b=PB))
            nc.vector.tensor_copy(
                sa_r[:, :, C:2 * C], zlast[:].rearrange("p (b c) -> p b c", b=PB))
            cur, nxt = (sc_a, sa_r), (sc_b, sb_r)
            for d in range(SCAN_LEVELS):
                step = SCAN_RADIX ** d
                ps = pss.tile([H, COLS], F32, tag="scan")
                for k in range(1, SCAN_RADIX):
                    sh = k * step
                    w = d * (SCAN_RADIX - 1) + (k - 1)
                    nc.tensor.matmul(
                        ps[:], wp_sb[:, w * H:(w + 1) * H],
                        cur[1][:, :, C - sh:2 * C - sh],
                        start=(k == 1), stop=(k == SCAN_RADIX - 1))
                nc.vector.tensor_add(nxt[1][:, :, C:2 * C],
                                     ps[:].rearrange("p (b c) -> p b c", b=PB),
                                     cur[1][:, :, C:2 * C])
                cur, nxt = nxt, cur
            xend_r = cur[1]          # (H, b, 2C): data at [C:2C], zeros before
            # xprev_c = xend_{c-1}: shift right by one chunk
            xprev = xend_r[:, :, C - 1:2 * C - 1]   # (H, PB, C)

            # ---- per-t: Z/X then y ----
            xt_sb = bigpool.tile([H, T * COLS], MM)
            yT = bigpool.tile([S, PB * L], MM)
            yT_r = yT[:].rearrange("p (b c t) -> p b c t", b=PB, c=C, t=T)
            for t in range(T):
                z = psz.tile([H, COLS], F32, tag="z")
                for s in range(t + 1):
                    nc.tensor.matmul(
                        z[:], gt_sb[:, (t - s) * H:(t - s + 1) * H], uT_r[:, :, :, s],
                        start=(s == 0), stop=False)
                nc.tensor.matmul(z[:], ap_sb[:, t * H:(t + 1) * H], xprev,
                                 start=False, stop=True)
                eng = nc.scalar.copy if t % 2 else nc.vector.tensor_copy
                eng(xt_sb[:, t * COLS:(t + 1) * COLS], z[:])

                yp = psy.tile([S, COLS], F32, tag="y")
                nc.tensor.matmul(yp[:], ct_sb[:], xt_sb[:, t * COLS:(t + 1) * COLS],
                                 start=True, stop=False)
                nc.tensor.matmul(yp[:], dd_sb[:], uT_r[:, :, :, t],
                                 start=False, stop=True)
                eng = nc.scalar.copy if t % 2 else nc.vector.tensor_copy
                eng(yT_r[:, :, :, t], yp[:].rearrange("p (b c) -> p b c", b=PB))

            # ---- transpose back & store:  y[b, l, o] ----
            # yT col = b*L + l  (already l-ordered)
            y_flat = y_d[:].rearrange("b l i -> (b l) i")
            for it in range(NTILE):
                pt = pst.tile([128, S], MM, tag="tp")
                nc.tensor.transpose(
                    pt[:], yT[:, it * 128:(it + 1) * 128], ident[:])
                ot = ostpool.tile([128, S], F32, tag="ostage")
                eng = nc.scalar.copy if it % 2 else nc.vector.tensor_copy
                eng(ot[:], pt[:])
                nc.sync.dma_start(y_flat[it * 128:(it + 1) * 128, :], ot[:])

    _split_multi_waits(nc)
    return nc


_NC_CACHE = {}


def _get_nc():
    if "nc" not in _NC_CACHE:
        _NC_CACHE["nc"] = _build()
    return _NC_CACHE["nc"]


def _host_precompute(Bmat, Cmat, Dvec, log_dt):
    Bm = np.asarray(Bmat, dtype=np.float64)
    x = np.float64(log_dt)
    dt = np.clip(np.logaddexp(0.0, x), DT_MIN, DT_MAX)   # softplus, clipped
    A = -np.eye(H) + np.eye(H, k=-1)
    back = np.eye(H) - 0.5 * dt * A
    fwd = np.eye(H) + 0.5 * dt * A
    dA = np.linalg.solve(back, fwd)
    dB = np.linalg.solve(back, dt * Bm)                  # (H, S)
    G = [dB]
    for _ in range(1, T):
        G.append(dA @ G[-1])
    dApow = [dA]
    for _ in range(1, T):
        dApow.append(dA @ dApow[-1])
    A8 = dApow[T - 1]
    Wp = []
    for d in range(SCAN_LEVELS):
        for k in range(1, SCAN_RADIX):
            Wp.append(np.linalg.matrix_power(A8, k * SCAN_RADIX ** d))
    f32 = lambda a: np.ascontiguousarray(a, dtype=np.float32)
    return {
        "GT": f32(np.stack([g.T for g in G])),                 # (T, S, H)
        "APOWT": f32(np.stack([p.T for p in dApow])),          # (T, H, H)
        "WPT": f32(np.stack([w.T for w in Wp])),               # (NWP, H, H)
        "CT2": f32(np.asarray(Cmat, dtype=np.float64).T),      # (H, S)
        "DD": f32(np.diag(np.asarray(Dvec, dtype=np.float64))),
        "EYE": f32(np.eye(128)),
        "ZZ": f32(np.zeros((H, COLS))),
    }


def kernel(u, B, C, D, log_dt, _trace=False):
    u = np.ascontiguousarray(u, dtype=np.float32)
    pre = _host_precompute(B, C, D, log_dt)
    nc = _get_nc()
    in_maps = [{"u": u[k * PB:(k + 1) * PB], **pre} for k in range(N_CORES)]
    res = run_bass_kernel_spmd(nc, in_maps, core_ids=list(range(N_CORES)),
                               trace=_trace)
    y = np.concatenate([res.results[k]["y"] for k in range(N_CORES)], axis=0)
    if _trace:
        kernel.last_result = res
    return y



# revision 40
# speedup vs baseline: 3.0587x; 3.0587x over previous
"""Trainium2 Bass kernel for nn_GammaSpaceLayer.

SSM with fixed "gamma" transition A (bidiagonal), bilinear discretization,
kernel k[m] = C dA^m dB, causal conv + D*u skip.  Computed as a chunked
linear scan (state dim H=64, chunk T=8), all in bf16 on-chip:

  per (batch, chunk c, local t):  x_t = sum_{s<=t} dA^{t-s} dB u[c,s]
                                        + dA^{t+1} xend[c-1]
  y_t = C x_t + D u_t

Structure per core (2 batches, processed as two interleaved "halves"):
  - u loaded+transposed in ONE XBAR DMA-transpose per batch (bf16).
  - zlast (chunk-local state at t=7) via 8 matmuls; radix-4 Hillis-Steele
    log-scan over the 256 chunk columns gives xend per chunk.
  - x_t for t=0..6 via a stride-2 state recursion
    (x_t = dA^2 x_{t-2} + dB u_t + dA dB u_{t-1}): 2 parallel short chains
    instead of 36 direct matmuls.  x_7 = xend directly; b0's x_6 is computed
    direct (u-terms prefilled into the PE-idle window while u1 loads).
  - y produced ROW-major by flipping matmul roles: the data (x / uT
    columns) is the stationary operand, the weights (C^T / diag(D)) are
    moving, so PSUM tiles come out [row, channel] and need no transpose.
  - ONE store DMA per batch.

Small input-dependent matrices are precomputed on host in float64, cast to
bf16 and passed as two packed constant blobs (NEFF cache friendly).
"""

import numpy as np
import ml_dtypes

import concourse.bass as bass
import concourse.mybir as mybir
import concourse.tile as tile
from concourse.vector_clock import ScopedClock
from concourse.bass_utils import run_bass_kernel_spmd

# problem constants (hardcoded per contract)
H, S = 64, 128          # state dim, io channel dim
B, L = 16, 2048         # full batch, seq len
N_CORES = 8
PB = B // N_CORES       # batches per core (2)
T = 8                   # chunk length
C = L // T              # chunks per batch (256)
DT_MIN, DT_MAX = 0.001, 0.1

F32 = mybir.dt.float32
BF = mybir.dt.bfloat16

# blob64 column offsets
W64_SCAN = 0            # 12 x [64,64] scan weights ((dA^8)^{k*4^d})^T
W64_APT0 = 768          # (dA^1)^T
W64_APT1 = 832          # (dA^2)^T
W64_CT2 = 896           # C^T  [64,128]
W64_APT6 = 1024         # (dA^7)^T
W64_COLS = 1088
WARMUP_MM = 18          # PE ramp warm-up matmuls (zeros, 256 cols each)
# blob128 column offsets
W128_GT = 0             # 8 x [128,64]  (dA^m dB)^T
W128_DD = 512           # diag(D) [128,128]
W128_COLS = 640


class _TC(tile.TileContext):
    """TileContext whose tail drain splits multi-sem waits: this walrus
    build caps CTRL instructions at one sync-wait command."""

    def _drain_and_barrier(self, tick_clock, wait_clock):
        probe = self.nc.sync.drain()
        wait_clock.add_sem_waits(probe.ins, ScopedClock({None: tick_clock.global_clock}))
        si = probe.ins.sync_info
        if si is not None and si.on_wait and len(si.on_wait) > 1:
            waits = list(si.on_wait)
            probe.ins.sync_info = mybir.SyncInfo(
                on_wait=[waits[0]], on_update=list(si.on_update or []))
            for w in waits[1:]:
                d = self.nc.sync.drain()
                d.ins.sync_info = mybir.SyncInfo(on_wait=[w], on_update=[])
        self.nc.all_engine_barrier()
        assert self.sems is not None
        popped = self.nc._tile_sem_poison_stack.pop()
        assert popped is self._sem_poison
        self.nc.clear_and_free_semaphores(list(self.sems.allocated().values()))
        self.nc.all_engine_barrier()


def _split_multi_waits(nc):
    """This walrus build allows only ONE sync-wait command per instruction.
    Split extras onto same-engine InstEventSemaphore carriers inserted
    immediately before (engine program order preserves semantics)."""
    n = 0
    for f in nc.m.functions:
        for b in f.blocks:
            il = b.instructions
            i = 0
            while i < len(il):
                ins = il[i]
                si = ins.sync_info
                if si is not None and si.on_wait and len(si.on_wait) > 1:
                    waits = list(si.on_wait)
                    ins.sync_info = mybir.SyncInfo(
                        on_wait=[waits[-1]], on_update=list(si.on_update or []))
                    for j, w in enumerate(waits[:-1]):
                        ev = mybir.InstEventSemaphore(
                            name=f"{ins.name}_wsplit{j}", ins=[], outs=[])
                        ev.engine = ins.engine
                        ev.sync_info = mybir.SyncInfo(on_wait=[w], on_update=[])
                        il.insert(i, ev)
                        i += 1
                        n += 1
                i += 1
    return n


def _build():
    nc = bass.Bass()
    u_d = nc.dram_tensor("u", [PB, L, S], BF, kind="ExternalInput")
    b128_d = nc.dram_tensor("B128", [W128_COLS, S], BF, kind="ExternalInput")
    b64_d = nc.dram_tensor("B64", [W64_COLS, S], BF, kind="ExternalInput")
    y_d = nc.dram_tensor("y", [PB, L, S], BF, kind="ExternalOutput")

    u_flat = u_d[:].rearrange("b l i -> (b l) i")          # [4096, 128]
    y_pr = y_d[:].rearrange("b (p j) o -> (b p) (j o)", p=128, j=16)  # [256, 2048]

    with _TC(nc) as tc:
        with (
            tc.tile_pool(name="const", bufs=1) as cpool,
            tc.tile_pool(name="big", bufs=1) as bigpool,
            tc.tile_pool(name="psz", bufs=3, space="PSUM") as psz,
            tc.tile_pool(name="pssc", bufs=2, space="PSUM") as pssc,
            tc.tile_pool(name="psy", bufs=3, space="PSUM") as psy,
        ):
            w128 = cpool.tile([S, W128_COLS], BF)
            # 64-partition weights ride a 128-partition XBAR blob; partitions
            # 64..128 are zero padding (matmul needs operand base partitions
            # to match, so everything stays based at partition 0).
            w64b = cpool.tile([S, W64_COLS], BF)
            warm = cpool.tile([S, 256], BF)
            uT = [bigpool.tile([S, L], BF, tag=f"uT{b}", name=f"uT{b}") for b in range(PB)]
            sc_a = [bigpool.tile([H, 2 * C], BF, tag=f"sca{b}", name=f"sca{b}") for b in range(PB)]
            sc_b = [bigpool.tile([H, 2 * C], BF, tag=f"scb{b}", name=f"scb{b}") for b in range(PB)]
            xq = [bigpool.tile([H, C + 2], BF, tag=f"xq{b}", name=f"xq{b}") for b in range(PB)]
            x_sb = [bigpool.tile([H, L], BF, tag=f"x{b}", name=f"x{b}") for b in range(PB)]
            yst = [bigpool.tile([S, L], BF, tag=f"yst{b}", name=f"yst{b}") for b in range(PB)]

            # ---- input DMAs: ALL XBAR-transpose kind on the SP queue ----
            # Mixing plain and XBAR DMAs (or engines) makes the scheduler
            # chain completion waits at every kind-crossing (queue-set swap);
            # a homogeneous XBAR stream rides the FIFO with no waits.  The
            # const blobs are stored pre-transposed in DRAM for this.
            nc.sync.dma_start(w128[:], b128_d[:], transpose=True)
            nc.sync.dma_start(uT[0][:], u_flat[0:L, :], transpose=True)
            nc.sync.dma_start(uT[1][:], u_flat[L:2 * L, :], transpose=True)
            nc.sync.dma_start(w64b[:], b64_d[:], transpose=True)
            # zero pads (scan shift-in region, xprev shift-in column)
            for b in range(PB):
                nc.gpsimd.memset(sc_a[b][:, 0:C], 0.0)
                nc.gpsimd.memset(sc_b[b][:, 0:C], 0.0)
                nc.vector.memset(xq[b][:, 0:1], 0.0)

            # PE warm-up: zero matmuls keep the tensor engine continuously
            # busy from ~1.4us so the p-state ramp (3us continuous -> 2.4GHz)
            # completes before the first real matmul.
            nc.vector.memset(warm[:], 0.0)
            wps = psy.tile([S, 4 * S], F32, tag="y", name="warmps")
            for i in range(WARMUP_MM):
                nc.tensor.matmul(wps[:, 0:C], warm[:, 0:S], warm[:],
                                 start=(i == 0), stop=(i == WARMUP_MM - 1))

            # s-slices of uT: [128, 256] stride-8 APs
            uTr = [uT[b][:].rearrange("p (c t) -> p c t", t=T) for b in range(PB)]
            x_r = [x_sb[b][:].rearrange("p (c t) -> p c t", t=T) for b in range(PB)]
            # stationary slices for y tiles: cols {16*pp + j}
            uTj = [uT[b][:].rearrange("p (pp j) -> p j pp", j=16) for b in range(PB)]
            x_j = [x_sb[b][:].rearrange("p (pp j) -> p j pp", j=16) for b in range(PB)]

            def gt(m):                      # (dA^m dB)^T  [128, 64]
                return w128[:, W128_GT + m * H:W128_GT + (m + 1) * H]

            def w64(c0, c1):                # [64, W64_COLS] weight accessor
                return w64b[0:H, c0:c1]

            # ---- zlast: chunk-local state at t = T-1 ----
            zl = []
            ps0 = psz.tile([H, C], F32, tag="z", name="zl0")
            for s in range(T):
                nc.tensor.matmul(ps0, gt(T - 1 - s), uTr[0][:, :, s],
                                 start=(s == 0), stop=(s == T - 1))
            nc.scalar.copy(sc_a[0][:, C:2 * C], ps0[:])
            # direct-t6 u-terms for b0: fills PE while zlast_b1 waits on u1
            z6_0 = psz.tile([H, C], F32, tag="z", name="z6_0")
            for s in range(7):
                nc.tensor.matmul(z6_0, gt(6 - s), uTr[0][:, :, s],
                                 start=(s == 0), stop=False)
            ps1 = psz.tile([H, C], F32, tag="z", name="zl1")
            for s in range(T):
                nc.tensor.matmul(ps1, gt(T - 1 - s), uTr[1][:, :, s],
                                 start=(s == 0), stop=(s == T - 1))
            nc.scalar.copy(sc_a[1][:, C:2 * C], ps1[:])

            # ---- radix-4 log-scan over chunks (interleave the two halves) ----
            cur = [sc_a[0], sc_a[1]]
            nxt = [sc_b[0], sc_b[1]]
            for d in range(4):
                step = 4 ** d
                for b in range(PB):
                    ps = pssc.tile([H, C], F32, tag="scan")
                    for k in range(1, 4):
                        sh = k * step
                        w = (3 * d + k - 1) * H
                        nc.tensor.matmul(ps, w64(W64_SCAN + w, W64_SCAN + w + H),
                                         cur[b][:, C - sh:2 * C - sh],
                                         start=(k == 1), stop=(k == 3))
                    dst = xq[b][:, 1:C + 1] if d == 3 else nxt[b][:, C:2 * C]
                    nc.vector.tensor_add(dst, ps[:], cur[b][:, C:2 * C])
                cur, nxt = nxt, cur

            # x_7 = xend: y t=7 tiles read xq directly (stride-2 APs), so
            # the t=7 slice of x is never materialized
            xqr = [xq[b][:].rearrange("p (c two) -> p two c", two=2)
                   for b in range(PB)]

            # ---- chain + y tiles, interleaved ----
            apt0 = w64(W64_APT0, W64_APT0 + H)
            apt6 = w64(W64_APT6, W64_APT6 + H)
            apt1 = w64(W64_APT1, W64_APT1 + H)
            ct2 = w64(W64_CT2, W64_CT2 + S)
            dd = w128[:, W128_DD:W128_DD + S]
            # GPSIMD cannot read PSUM on real HW: PSUM-source copies are
            # restricted to Activation + DVE.
            cp_eng = [nc.scalar.copy, nc.vector.tensor_copy]

            def chain_step(b, t):
                if b == 0 and t == 6:
                    nc.tensor.matmul(z6_0, apt6, xq[0][:, 0:C],
                                     start=False, stop=True)
                    cp_eng[t % 2](x_r[0][:, :, 6], z6_0[:])
                    return
                ps = psz.tile([H, C], F32, tag="z")
                nc.tensor.matmul(ps, gt(0), uTr[b][:, :, t], start=True, stop=False)
                if t == 0:
                    nc.tensor.matmul(ps, apt0, xq[b][:, 0:C], start=False, stop=True)
                else:
                    nc.tensor.matmul(ps, gt(1), uTr[b][:, :, t - 1],
                                     start=False, stop=False)
                    src = xq[b][:, 0:C] if t == 1 else x_r[b][:, :, t - 2]
                    nc.tensor.matmul(ps, apt1, src, start=False, stop=True)
                cp_eng[(b + t) % 2](x_r[b][:, :, t], ps[:])

            # 4D store views: row l = 16p + 8jh + jl -> j = 8jh + jl, t = j%8
            yst_r4 = [yst[b][:].rearrange("p (jh jl o) -> p jh jl o", jh=2, jl=8)
                      for b in range(PB)]
            y_pr4 = [y_pr[b * 128:(b + 1) * 128, :]
                     .rearrange("p (jh jl o) -> p jh jl o", jh=2, jl=8)
                     for b in range(PB)]
            yc = 0

            yst_j = [yst[b][:].rearrange("p (jh jl o) -> p jh jl o", jh=2, jl=8)
                     for b in range(PB)]

            def y_mms(yp, k, b, t, j):
                sl = yp[:, k * S:(k + 1) * S]
                if t == 7:
                    xs = xqr[b][:, 1, 0:128] if j < 8 else xqr[b][:, 0, 1:129]
                else:
                    xs = x_j[b][:, j]
                nc.tensor.matmul(sl, xs, ct2, start=True, stop=False)
                nc.tensor.matmul(sl, uTj[b][:, j], dd, start=False, stop=True)

            def y_tiles(b, t):
                # tiles j = t and t+8 finish together: one paired PSUM bank,
                # one strided copy for both (halves per-copy overhead)
                nonlocal yc
                yp = psy.tile([S, 4 * S], F32, tag="y")
                for k, j in enumerate((t, t + 8)):
                    y_mms(yp, k, b, t, j)
                cp_eng[yc % 2](yst_j[b][:, :, t, :],
                               yp[:, 0:2 * S].rearrange("p (jh o) -> p jh o", jh=2))
                yc += 1

            def y_quad(b, t0):
                # four tiles (t0, t0+1) x (jh) share one full PSUM bank and
                # leave in ONE copy: amortizes the fixed PSUM-access cost
                nonlocal yc
                yp = psy.tile([S, 4 * S], F32, tag="y")
                for k, j in enumerate((t0, t0 + 1, t0 + 8, t0 + 9)):
                    y_mms(yp, k, b, j % 8, j)
                cp_eng[yc % 2](
                    yst_j[b][:, :, t0:t0 + 2, :],
                    yp[:].rearrange("p (jh jl o) -> p jh jl o", jh=2, jl=2))
                yc += 1

            chain_step(0, 0)
            chain_step(0, 1)
            chain_step(0, 6)
            chain_step(1, 0)
            chain_step(1, 1)
            for b in range(PB):
                y_tiles(b, 7)
            # t=7 rows finish first: ship them immediately
            for b in range(PB):
                nc.sync.dma_start(y_pr4[b][:, :, 7:8, :], yst_r4[b][:, :, 7:8, :])
            for t in range(2, 7):
                if t < 6:
                    chain_step(0, t)
                if t in (3, 5):
                    y_quad(0, t - 3)
                chain_step(1, t)
                if t in (3, 5):
                    y_quad(1, t - 3)
            # after y t<=3 done for a batch, store those rows early (jl<4)
            nc.sync.dma_start(y_pr4[0][:, :, 0:4, :], yst_r4[0][:, :, 0:4, :])
            nc.sync.dma_start(y_pr4[1][:, :, 0:4, :], yst_r4[1][:, :, 0:4, :])
            for b in range(PB):
                y_quad(b, 4)
                y_tiles(b, 6)

            # ---- stores: part B per batch (t=4..6; t7 and t<=3 already sent) ----
            for b in range(PB):
                nc.sync.dma_start(y_pr4[b][:, :, 4:7, :], yst_r4[b][:, :, 4:7, :])

    _split_multi_waits(nc)
    return nc


_NC_CACHE = {}


def _get_nc():
    if "nc" not in _NC_CACHE:
        _NC_CACHE["nc"] = _build()
    return _NC_CACHE["nc"]


def _host_precompute(Bmat, Cmat, Dvec, log_dt):
    Bm = np.asarray(Bmat, dtype=np.float64)
    x = np.float64(log_dt)
    dt = np.clip(np.logaddexp(0.0, x), DT_MIN, DT_MAX)   # softplus, clipped
    A = -np.eye(H) + np.eye(H, k=-1)
    back = np.eye(H) - 0.5 * dt * A
    fwd = np.eye(H) + 0.5 * dt * A
    dA = np.linalg.solve(back, fwd)
    dB = np.linalg.solve(back, dt * Bm)                  # (H, S)
    G = [dB]
    for _ in range(1, T):
        G.append(dA @ G[-1])
    A8 = np.linalg.matrix_power(dA, T)

    bf = ml_dtypes.bfloat16
    b128 = np.zeros((S, W128_COLS), dtype=np.float64)
    for m in range(T):
        b128[:, W128_GT + m * H:W128_GT + (m + 1) * H] = G[m].T
    b128[:, W128_DD:W128_DD + S] = np.diag(np.asarray(Dvec, dtype=np.float64))

    b64 = np.zeros((H, W64_COLS), dtype=np.float64)
    for d in range(4):
        for k in range(1, 4):
            w = (3 * d + k - 1) * H
            b64[:, W64_SCAN + w:W64_SCAN + w + H] = \
                np.linalg.matrix_power(A8, k * 4 ** d).T
    b64[:, W64_APT0:W64_APT0 + H] = dA.T
    b64[:, W64_APT1:W64_APT1 + H] = (dA @ dA).T
    b64[:, W64_CT2:W64_CT2 + S] = np.asarray(Cmat, dtype=np.float64).T
    b64[:, W64_APT6:W64_APT6 + H] = np.linalg.matrix_power(dA, 7).T

    # both blobs stored TRANSPOSED so they ride the same XBAR DMA queue as
    # u; the 64-part blob is zero-padded to 128 partitions post-transpose
    b64t = np.zeros((W64_COLS, S), dtype=np.float64)
    b64t[:, :H] = b64.T
    return {
        "B128": np.ascontiguousarray(b128.T, dtype=bf),
        "B64": np.ascontiguousarray(b64t, dtype=bf),
    }


def kernel(u, B, C, D, log_dt):
    # (shadows the module-level chunk-count C inside this function only)
    u = np.ascontiguousarray(u, dtype=np.float32)
    pre = _host_precompute(B, C, D, log_dt)
    u_bf = u.astype(ml_dtypes.bfloat16)
    nc = _get_nc()
    in_maps = [{"u": u_bf[k * PB:(k + 1) * PB], **pre} for k in range(N_CORES)]
    res = run_bass_kernel_spmd(nc, in_maps, core_ids=list(range(N_CORES)))
    y = np.concatenate(
        [res.results[k]["y"].astype(np.float32) for k in range(N_CORES)], axis=0)
    return y
